# revision 32
# baseline (speedup 1.0000x reference)
"""Distributed multi-head attention kernel for one TRN2 chip (8 NeuronCores).

Problem: B=2, S=2048, D=1024, H=16 heads (head_dim 64), torch-style
Linear QKV projections + softmax attention + out projection.

Sharding: tensor-parallel over heads, 2 heads per core (all 8 cores see the
full batch).  Each core:
  1. computes qT/kT/vT = (x @ W.T + b).T for its 2 heads (E=128 local dims),
  2. runs softmax attention for its (2 heads x 2 batches) fully locally
     (scores computed transposed [k, q] so softmax-sum comes free via an
     appended ones-column in V during the PV matmul),
  3. AllToAll redistributes attention outputs so core j owns query rows
     [j*512:(j+1)*512] of the flattened [B*S, D] activations with all 16
     heads present,
  4. local out-projection (x @ Wo.T + bo) for its 512 rows.
Host reassembles the 8 row-blocks into the [B, S, D] output.

Scheduling: attention is ACT(exp)-bound, so PE-only work (batch-1
projections, batch-0 out-projection) is interleaved into the attention
instruction stream as fillers — this keeps the TensorEngine continuously
busy so the HAM clock gate stays at 2.4 GHz.

Compute dtype: bfloat16 on the matmuls (fp32 accumulation in PSUM), exp in
fp32->bf16 on the scalar engine, normalization in fp32 on the vector engine.
"""

import numpy as np

B = 2
S = 2048
D = 1024
H = 16
DH = 64
N_CORES = 8
HPC = H // N_CORES  # heads per core = 2
E = HPC * DH  # local head dims = 128
ROWS = B * S // N_CORES  # output rows per core = 512
NT = B * S  # total tokens = 4096
DCH = D // 128  # d-model chunks of 128 = 8
SCALE = 1.0 / np.sqrt(DH)

_CACHE = {}


def _bf16(x):
    import ml_dtypes

    return np.ascontiguousarray(x).astype(ml_dtypes.bfloat16)


def _build():
    """Build + compile the SPMD Bass graph (identical on all 8 cores)."""
    from concourse import bacc, tile, mybir

    bf16 = mybir.dt.bfloat16
    f32 = mybir.dt.float32
    AF = mybir.ActivationFunctionType

    nc = bacc.Bacc("TRN2", target_bir_lowering=False, debug=False, num_devices=N_CORES)

    # ---- I/O -----------------------------------------------------------
    # activations, pre-transposed AND pre-tiled on host:
    # [NT//512, 128, DCH, 512]: element (t, p, d, c) = x[t*512 + c, d*128 + p]
    xq = nc.dram_tensor("xq", [NT // 512, 128, DCH, 512], bf16, kind="ExternalInput")
    xk = nc.dram_tensor("xk", [NT // 512, 128, DCH, 512], bf16, kind="ExternalInput")
    xv = nc.dram_tensor("xv", [NT // 512, 128, DCH, 512], bf16, kind="ExternalInput")
    # weights, pre-transposed/sliced on host: [128, DCH, E]:
    # (p, d, e) = W[head_slice][e_global, d*128+p] (scale folded into wq)
    wq = nc.dram_tensor("wq", [128, DCH, E], bf16, kind="ExternalInput")
    wk = nc.dram_tensor("wk", [128, DCH, E], bf16, kind="ExternalInput")
    wv = nc.dram_tensor("wv", [128, DCH, E], bf16, kind="ExternalInput")
    # full Wo.T: (p, d, e) = Wo[e, d*128+p]
    wo = nc.dram_tensor("wo", [128, DCH, D], bf16, kind="ExternalInput")
    # biases: per-partition columns (scale folded into bq)
    bq = nc.dram_tensor("bq", [128, 1], f32, kind="ExternalInput")
    bk = nc.dram_tensor("bk", [128, 1], f32, kind="ExternalInput")
    bv = nc.dram_tensor("bv", [128, 1], f32, kind="ExternalInput")
    # bo replicated across partitions
    bo = nc.dram_tensor("bo", [128, D], f32, kind="ExternalInput")
    out = nc.dram_tensor("out", [ROWS, D], f32, kind="ExternalOutput")

    SKT = S // 128  # 16 k-tiles per batch

    with tile.TileContext(nc) as tc:
        with (
            tc.tile_pool(name="dram", bufs=1, space="DRAM") as dram,
            tc.tile_pool(name="wpool", bufs=1) as wpool,
            tc.tile_pool(name="xs", bufs=6) as xs_pool,
            tc.tile_pool(name="qk", bufs=1) as qk_pool,
            tc.tile_pool(name="vpool", bufs=1) as v_pool,
            # PSUM budget (8 banks): scores/proj/outproj share 2x[128,1024]
            # slots = 4 banks, double-buffered PV accumulators = 4 banks
            tc.tile_pool(name="ps", bufs=2, space="PSUM") as ps_pool,
            tc.tile_pool(name="pso", bufs=2, space="PSUM") as pso_pool,
            tc.tile_pool(name="ex", bufs=34) as e_pool,
            tc.tile_pool(name="norm", bufs=2) as n_pool,
            tc.tile_pool(name="ao", bufs=1) as ao_pool,
            tc.tile_pool(name="outp", bufs=2) as out_pool,
        ):
            a2a_in = [dram.tile([N_CORES, E, ROWS // 2], bf16, name=f"a2ai{b}")
                      for b in range(B)]
            a2a_out = [dram.tile([N_CORES, E, ROWS // 2], bf16, name=f"a2ao{b}")
                       for b in range(B)]
            warm_in = dram.tile([N_CORES, 128], bf16, name="warm_in")
            warm_out = dram.tile([N_CORES, 128], bf16, name="warm_out")

            # ---- load weights / biases --------------------------------
            wq_sb = wpool.tile([128, DCH, E], bf16, tag="wq")
            wk_sb = wpool.tile([128, DCH, E], bf16, tag="wk")
            wv_sb = wpool.tile([128, DCH, E], bf16, tag="wv")
            wo_sb = wpool.tile([128, DCH, D], bf16, tag="wo")
            bq_sb = wpool.tile([128, 1], f32, tag="bq")
            bk_sb = wpool.tile([128, 1], f32, tag="bk")
            bv_sb = wpool.tile([128, 1], f32, tag="bv")
            bo_sb = wpool.tile([128, D], f32, tag="bo")
            ident = wpool.tile([128, 128], bf16, tag="ident")
            import ml_dtypes

            ident_dram = nc.inline_tensor(
                np.eye(128, dtype=ml_dtypes.bfloat16), name="ident_c"
            )
            for sb, dr in [
                (ident, ident_dram),
                (wq_sb, wq), (wk_sb, wk), (wv_sb, wv),
                (bq_sb, bq), (bk_sb, bk), (bv_sb, bv),
            ]:
                nc.scalar.dma_start(sb[:], dr[:])

            # ---- persistent activation tiles --------------------------
            qT = [qk_pool.tile([128, S], bf16, tag=f"qT{b}", name=f"qT{b}")
                  for b in range(B)]
            kT = [qk_pool.tile([128, S], bf16, tag=f"kT{b}", name=f"kT{b}")
                  for b in range(B)]
            vT = [qk_pool.tile([128, S], bf16, tag=f"vT{b}", name=f"vT{b}")
                  for b in range(B)]
            # v natural, augmented with ones col: [s-part, kt, h, 65]
            v_sb = [v_pool.tile([128, SKT, HPC, DH + 1], bf16, tag=f"v{b}",
                                name=f"v{b}") for b in range(B)]

            # collective warmup: a tiny AllToAll during the projection
            # phase absorbs the first-call ncfw setup cost (~8us) so the
            # real collectives run at steady-state latency
            nc.gpsimd.collective_compute(
                "AllToAll",
                mybir.AluOpType.bypass,
                replica_groups=[list(range(N_CORES))],
                ins=[warm_in[:].opt()],
                outs=[warm_out[:].opt()],
            )

            # PE warmup: ~5us of dummy transposes so HAM un-throttles
            # before the first projection matmuls arrive.
            wps = ps_pool.tile([128, 512], bf16, tag="ps", name="warm")
            for _ in range(44):
                nc.tensor.transpose(wps[:, 0:128], ident[:], ident[:])

            def junk_mms(n):
                # discarded matmuls on resident tiles: pure PE-activity
                # filler so the HAM clock gate stays at 2.4 GHz while the
                # scalar engine paces attention
                jt = ps_pool.tile([128, 512], f32, tag="ps", name="junk")
                for _ in range(n):
                    nc.tensor.matmul(jt[:], ident[:], wo_sb[:, 0, 0:512],
                                     start=True, stop=True)

            def proj_tile(xdram, w_sb, bias_sb, out_tile, tg, st, qi=[0]):
                # one 512-column projection tile: DMA + 8 matmuls + bias.
                # First 4 tiles stay on the sync queue (the warmup
                # collective's wait occupies gpsimd for ~10us at t=0);
                # later tiles alternate queues to double delivery bandwidth.
                xt = xs_pool.tile([128, DCH, 512], bf16, tag="xt")
                qi[0] += 1
                q = nc.sync if (qi[0] <= 4 or qi[0] % 2 == 0) else nc.gpsimd
                q.dma_start(xt[:], xdram[tg])
                ps = ps_pool.tile([128, 512], f32, tag="ps", name="ps_proj")
                for d in range(DCH):
                    nc.tensor.matmul(
                        ps[:], w_sb[:, d, :], xt[:, d, :],
                        start=(d == 0), stop=(d == DCH - 1),
                    )
                nc.vector.tensor_scalar_add(
                    out_tile[:, st * 512:(st + 1) * 512], ps[:], bias_sb[:]
                )

            def proj_thunks(b):
                th = []
                for xdram, w_sb, bias_sb, out_t in (
                    (xq, wq_sb, bq_sb, qT[b]),
                    (xk, wk_sb, bk_sb, kT[b]),
                    (xv, wv_sb, bv_sb, vT[b]),
                ):
                    for st in range(4):
                        th.append(
                            lambda xd=xdram, w=w_sb, bi=bias_sb, o=out_t, \
                                   tg=b * 4 + st, s=st: proj_tile(xd, w, bi, o, tg, s)
                        )
                return th

            def v_finish(b):
                # natural-layout V (with ones column) via PE transposes
                nc.vector.memset(v_sb[b][:, :, :, DH:DH + 1], 1.0)
                for c in range(SKT):
                    pst = ps_pool.tile([128, 512], bf16, tag="ps", name="pst")
                    nc.tensor.transpose(
                        pst[:, 0:128], vT[b][:, c * 128:(c + 1) * 128], ident[:]
                    )
                    nc.vector.tensor_copy(
                        v_sb[b][:, c, :, 0:DH],
                        pst[:, 0:128].rearrange("p (h d) -> p h d", h=HPC),
                    )

            # ---- attention (head-sequential, fillers interleaved) -----
            def attention_batch(b, fillers):
                # Unit (qh, h) = phase A (16 score tiles + exp) then phase B
                # (32 PV matmuls). Units are software-pipelined: phase A of
                # unit n+1 is emitted before phase B of unit n, so the scalar
                # engine exps unit n+1 while the PE runs unit n's PVs — both
                # engines stay continuously busy.
                aoT = [ao_pool.tile([64, S], bf16, tag=f"aoT{h}",
                                    name=f"aoT{b}_{h}") for h in range(HPC)]
                exs_store = {}

                def phase_a(qh, h):
                    q0 = qh * 1024
                    p0 = h * 64
                    exs = []
                    for c in range(SKT):
                        pss = ps_pool.tile([128, 1024], f32, tag="ps",
                                           name="pss")
                        for half in range(2):
                            nc.tensor.matmul(
                                pss[:, half * 512:(half + 1) * 512],
                                kT[b][p0:p0 + 64, c * 128:(c + 1) * 128],
                                qT[b][p0:p0 + 64,
                                      q0 + half * 512:q0 + half * 512 + 512],
                                start=True, stop=True,
                                tile_position=(p0, 0),
                            )
                        ex = e_pool.tile([128, 1024], bf16, tag="ex",
                                         name=f"ex{c}")
                        nc.scalar.activation(ex[:], pss[:], AF.Exp)
                        exs.append(ex)
                    exs_store[(qh, h)] = exs

                def phase_b(qh, h):
                    q0 = qh * 1024
                    exs = exs_store.pop((qh, h))
                    pso = pso_pool.tile([65, 1024], f32, tag="pso",
                                        name=f"pso{b}_{qh}_{h}")
                    for c in range(SKT):
                        for sub in range(2):
                            nc.tensor.matmul(
                                pso[:, sub * 512:(sub + 1) * 512],
                                v_sb[b][:, c, h, :],
                                exs[c][:, sub * 512:(sub + 1) * 512],
                                start=(c == 0), stop=(c == SKT - 1),
                            )
                    # PE-only filler work rides the exp shadow
                    if fillers:
                        for th in fillers.pop(0):
                            th()
                    # normalize
                    rc = n_pool.tile([1, 1024], f32, tag="rc")
                    nc.vector.reciprocal(rc[:], pso[64:65, :])
                    bc = n_pool.tile([64, 1024], f32, tag="bc")
                    nc.gpsimd.partition_broadcast(bc[:], rc[:])
                    nc.vector.tensor_mul(
                        aoT[h][:, q0:q0 + 1024], pso[0:64, :], bc[:]
                    )
                    # ship this (q-half, head) slice immediately: q-half qh
                    # covers shards 4qh..4qh+3 (q rows j*256..)
                    nc.scalar.dma_start(
                        a2a_in[b][qh * 4:(qh + 1) * 4,
                                  h * 64:(h + 1) * 64, :]
                        .transpose([1, 0, 2]),
                        aoT[h][:, q0:q0 + 1024]
                        .rearrange("p (j c) -> p j c", j=4),
                    )

                units = [(qh, h) for qh in range(2) for h in range(HPC)]
                phase_a(*units[0])
                for i in range(len(units)):
                    if i + 1 < len(units):
                        phase_a(*units[i + 1])
                    phase_b(*units[i])

            def a2a(b):
                nc.gpsimd.collective_compute(
                    "AllToAll",
                    mybir.AluOpType.bypass,
                    replica_groups=[list(range(N_CORES))],
                    ins=[a2a_in[b][:].opt()],
                    outs=[a2a_out[b][:].opt()],
                )

            # ---- out projection (per batch half: 256 rows) -----------
            def outproj_group(b, ao_d, st, half):
                e0 = half * 512
                ps = ps_pool.tile([128, 512], f32, tag="ps", name="ps_out")
                for d in range(DCH):
                    nc.tensor.matmul(
                        ps[:],
                        ao_d[d][:, st * 128:(st + 1) * 128],
                        wo_sb[:, d, e0:e0 + 512],
                        start=(d == 0), stop=(d == DCH - 1),
                    )
                ot = out_pool.tile([128, 512], f32, tag="ot")
                nc.vector.tensor_add(ot[:], ps[:], bo_sb[:, e0:e0 + 512])
                r0 = b * 256 + st * 128
                nc.sync.dma_start(out[r0:r0 + 128, e0:e0 + 512], ot[:])

            def outproj_thunks(b):
                # per-peer-chunk tiles: each 64KB load is an independent
                # dependency, so the first matmuls start as soon as the
                # first chunk lands instead of after the full 0.5MB
                ao_d = [ao_pool.tile([128, ROWS // 2], bf16, tag=f"ao_d{d}",
                                     name=f"ao{b}_d{d}") for d in range(DCH)]
                for d in range(DCH):
                    nc.sync.dma_start(ao_d[d][:], a2a_out[b][d])
                return [
                    lambda st=st, half=half: outproj_group(b, ao_d, st, half)
                    for st in range(2) for half in range(2)
                ]

            # ---- main flow -------------------------------------------
            for th in proj_thunks(0):
                th()
            # out-projection weights are needed only ~200us in; loading
            # them after the batch-0 projection emission keeps the 2.5MB
            # off the HBM-critical start window
            nc.scalar.dma_start(wo_sb[:], wo[:])
            nc.scalar.dma_start(bo_sb[:], bo[:])
            v_finish(0)
            # batch-1 projections ride inside attention(0): 3 tiles per slot
            p1 = proj_thunks(1)
            fill0 = [p1[i * 3:(i + 1) * 3] for i in range(4)]
            attention_batch(0, fill0)
            a2a(0)
            v_finish(1)
            # batch-0 out-projection: 2 groups ride inside attention(1),
            # 2 groups cover the final AllToAll; junk matmuls top up the
            # PE so it never idles long enough to re-throttle
            op0 = outproj_thunks(0)
            fill1 = [
                [op0[0], lambda: junk_mms(8)],
                [op0[1], lambda: junk_mms(8)],
                [lambda: junk_mms(12)],
                [lambda: junk_mms(12)],
            ]
            attention_batch(1, fill1)
            a2a(1)
            op0[2]()
            op0[3]()
            # keep the PE clock warm through the final collective wait so
            # the last out-projection runs at full rate
            junk_mms(24)
            for th in outproj_thunks(1):
                th()

    nc.compile()
    return nc


def _prep_inputs(query, key, value, Wq, bq, Wk, bk, Wv, bv, Wo, bo):
    """Host-side sharding/layout. Returns list of 8 per-core input dicts."""
    x_flat = {}
    for name, x in (("xq", query), ("xk", key), ("xv", value)):
        # [B,S,D] -> [NT, D] -> T [D, NT] -> [NT//512, 128, DCH, 512]
        xt = x.reshape(NT, D).T.reshape(DCH, 128, NT // 512, 512)
        x_flat[name] = _bf16(xt.transpose(2, 1, 0, 3))

    wo_l = _bf16(Wo.T.reshape(DCH, 128, D).transpose(1, 0, 2))
    bo_l = np.ascontiguousarray(
        np.broadcast_to(bo.astype(np.float32), (128, D))
    )

    in_maps = []
    for i in range(N_CORES):
        r0 = i * E  # global head-dim slice for this core
        m = dict(x_flat)
        m["wq"] = _bf16(
            (Wq[r0:r0 + E, :] * SCALE).T.reshape(DCH, 128, E).transpose(1, 0, 2)
        )
        m["wk"] = _bf16(Wk[r0:r0 + E, :].T.reshape(DCH, 128, E).transpose(1, 0, 2))
        m["wv"] = _bf16(Wv[r0:r0 + E, :].T.reshape(DCH, 128, E).transpose(1, 0, 2))
        m["wo"] = wo_l
        m["bq"] = np.ascontiguousarray(
            (bq[r0:r0 + E] * SCALE).astype(np.float32).reshape(128, 1)
        )
        m["bk"] = np.ascontiguousarray(bk[r0:r0 + E].astype(np.float32).reshape(128, 1))
        m["bv"] = np.ascontiguousarray(bv[r0:r0 + E].astype(np.float32).reshape(128, 1))
        m["bo"] = bo_l
        in_maps.append(m)
    return in_maps


def _get_nc():
    if "nc" not in _CACHE:
        _CACHE["nc"] = _build()
    return _CACHE["nc"]


def kernel(query, key, value, Wq, bq, Wk, bk, Wv, bv, Wo, bo, _trace=False):
    from concourse import bass_utils

    query = np.asarray(query, np.float32)
    key = np.asarray(key, np.float32)
    value = np.asarray(value, np.float32)
    nc = _get_nc()
    in_maps = _prep_inputs(
        query, key, value,
        np.asarray(Wq, np.float32), np.asarray(bq, np.float32),
        np.asarray(Wk, np.float32), np.asarray(bk, np.float32),
        np.asarray(Wv, np.float32), np.asarray(bv, np.float32),
        np.asarray(Wo, np.float32), np.asarray(bo, np.float32),
    )
    res = bass_utils.run_bass_kernel_spmd(
        nc, in_maps, core_ids=list(range(N_CORES)), trace=_trace
    )
    outf = np.empty((B, S, D), np.float32)
    half = ROWS // 2
    for i in range(N_CORES):
        o = res.results[i]["out"]
        for b in range(B):
            outf[b, i * half:(i + 1) * half] = o[b * half:(b + 1) * half]
    result = outf
    if _trace:
        _CACHE["last_results"] = res
    return result
